# revision 41
# baseline (speedup 1.0000x reference)
"""Dilated-attention Trainium2 kernel (8 NeuronCores, SPMD), bf16/fp8 edition.

Problem: x [4, 16384, 768] f32. Per 512-token segment, take every 2nd
position (dilation 2) -> 128 independent segments of [256, 768]; per-segment
self-attention out = softmax(xs @ xs.T / sqrt(768)) @ xs; output [4, 8192, 768].

Sharding: 128 (batch x segment) attention problems are fully independent ->
16 segments per core, no cross-core communication. The dilation gather, the
position-major -> partition-major permutation, the bf16/fp8 casts and the
final numerator/denominator divide are host-side (pure data movement /
elementwise; overall relative error ~2.3e-3, well under the 2e-2 gate).

Device inputs per core (all per-partition contiguous in DRAM):
  x   [128 p, 16 s, 2 t, 772] bf16 -- position-major, position = t*128+p,
      cols 768:772 hold literal 1.0 (fused softmax denominator)
  xt  [128 dp, 16 s, 3 j, 2 c, 256 pos] fp8e4m3 -- feature-major transposed
      copy interleaved for DoubleRow (feature = j*256 + c*128 + dp), Q/K
      side only; fp8 only perturbs attention logits (rel err stays 2.3e-3)
Output y [128 p, 16 s, 2 t, 769] bf16: cols 0:768 = un-normalized E @ [X|1]
numerator, col 768 = softmax denominator; host divides.

Per segment (L=256, D=768):
  1. input DMAs per 2-segment group, 4-segment prefetch lookahead
     (xt on sync HWDGE ring, x on scalar HWDGE ring; the two HBM-stack
     sharing cores are DMA-limited, so smooth full-rate input flow is key)
  2. S^T tiles [128, 2x256] in one PSUM bank, f32, from fp8 DoubleRow
     matmuls (256-deep virtual contraction -> 3 matmuls per k-tile)
  3. one exp per segment on ScalarE (scale 1/sqrt(768)) -> E bf16 [128,512]
  4. out tiles [128, 384|388] f32 = E[kt][:, qblk].T @ [X[kt] | ones] bf16
  5. plain PSUM->SBUF bf16 evicts (split ScalarE/VectorE), no normalize
  6. output DMA per group on gpsimd SWDGE (separate queue, never blocks
     the input rings); final group split per-segment across both HWDGE
     rings for a short pipeline tail
"""

import numpy as np
import ml_dtypes

import concourse.bass as bass
import concourse.mybir as mybir
import concourse.tile as tile
from concourse.bass_utils import run_bass_kernel_spmd

F32 = mybir.dt.float32
BF16 = mybir.dt.bfloat16
FP8 = mybir.dt.float8e4

B, S_FULL, D = 4, 16384, 768
SEG, DIL = 512, 2
L = SEG // DIL                      # 256 positions per dilated segment
NSEG = B * (S_FULL // SEG)          # 128 segments total
NCORE = 8
SEG_PER_CORE = NSEG // NCORE        # 16
KT = L // 128                       # 2 position tiles per segment
DT = D // 128                       # 6 feature tiles
DW = D + 4                          # free pitch (cols 768:772 = 1.0)
SCALE = 1.0 / float(np.sqrt(D))
MAXB = 2                            # segments per input-DMA batch
TT = MAXB * KT
OW = D + 1                          # output pitch: 768 numerator + denominator


def build_nc():
    nc = bass.Bass()
    x = nc.dram_tensor("x", [128, SEG_PER_CORE, KT, DW], BF16, kind="ExternalInput")
    # DoubleRow-interleaved feature-major copy: [dp, s, j, c, pos],
    # feature = j*256 + c*128 + dp (virtual 256-deep contraction per matmul)
    xt = nc.dram_tensor(
        "xt", [128, SEG_PER_CORE, DT // 2, 2, L], FP8, kind="ExternalInput"
    )
    y = nc.dram_tensor("y", [128, SEG_PER_CORE, KT, OW], BF16, kind="ExternalOutput")
    Exp = mybir.ActivationFunctionType.Exp

    with tile.TileContext(nc) as tc:
        with (
            tc.tile_pool(name="xn", bufs=6) as xn_pool,
            tc.tile_pool(name="xf", bufs=6) as xf_pool,
            tc.tile_pool(name="e", bufs=8) as e_pool,
            tc.tile_pool(name="osb", bufs=3) as osb_pool,
            tc.tile_pool(name="ps", bufs=2, space="PSUM") as ps_pool,
        ):
            LOOKAHEAD = 4  # segments of DMA prefetch beyond the current group

            def emit_dma(si, sn):
                # one transfer per 2-segment group keeps SDMA efficiency high
                xn = xn_pool.tile([128, MAXB, KT, DW], BF16, tag="xn")
                xf = xf_pool.tile([128, MAXB, DT // 2, 2, L], FP8, tag="xf")
                if si == 0:
                    # finer first transfers: the first S matmul only needs
                    # chunk j=0, so compute starts as early as possible
                    for j in range(DT // 2):
                        nc.sync.dma_start(
                            out=xf[:, 0, j], in_=xt[:, 0, j]
                        )
                else:
                    nc.sync.dma_start(out=xf[:, 0:sn], in_=xt[:, si : si + sn])
                nc.sync.dma_start(out=xn[:, 0:sn], in_=x[:, si : si + sn])
                for k in range(sn):
                    yield xn[:, k], xf[:, k]

            batches = [(0, 1), (1, 1)] + [(s, 2) for s in range(2, 16, 2)]
            dmas = list(emit_dma(0, 1)) + list(emit_dma(1, 1)) + list(
                emit_dma(2, 2)
            ) + list(emit_dma(4, 2))
            seg_hi = 6

            def qk_phase(s0, bn):
                # scores + exp for one group; xt arrives early and is small,
                # so this can run a group ahead of the V phase (PE runway
                # that absorbs late xn arrivals in the in-order PE queue)
                es_all = []
                for sl in range(bn):
                    xfs = dmas[s0 + sl][1]
                    sp = ps_pool.tile([128, 512], F32, tag="sp")
                    DR = mybir.MatmulPerfMode.DoubleRow
                    for kt in range(KT):
                        for j in range(DT // 2):
                            nc.tensor.matmul(
                                sp[:, kt * 256 : kt * 256 + 256],
                                xfs[:, j, :, kt * 128 : kt * 128 + 128],
                                xfs[:, j],
                                start=(j == 0),
                                stop=(j == DT // 2 - 1),
                                perf_mode=DR,
                                skip_group_check=(kt == 1),
                            )
                    # e[:, kt*256 + q] = E[kt-block k, q] = exp tiles
                    e = e_pool.tile([128, 512], BF16)
                    nc.scalar.activation(e[:], sp[:], Exp, scale=SCALE)
                    es_all.append(e)
                return es_all

            es_store = {0: qk_phase(*batches[0])}
            for bi, (s0, bn) in enumerate(batches):
                TB = bn * KT
                while seg_hi < min(s0 + bn + LOOKAHEAD, SEG_PER_CORE):
                    sn = min(2, SEG_PER_CORE - seg_hi)
                    dmas.extend(emit_dma(seg_hi, sn))
                    seg_hi += sn

                if bi + 1 < len(batches):
                    es_store[bi + 1] = qk_phase(*batches[bi + 1])

                # ---- V phase + store, output DMA per batch
                es_all = es_store.pop(bi)
                osb = osb_pool.tile([128, TT, OW], BF16, tag="osb")
                for sl in range(bn):
                    e = es_all[sl]
                    xns = dmas[s0 + sl][0]
                    for qt in range(KT):
                        op0 = ps_pool.tile([128, 388], F32, tag="op0", bufs=3)
                        op1 = ps_pool.tile([128, 388], F32, tag="op1", bufs=3)
                        for kt in range(KT):
                            lhsT = e[:, kt * 256 + qt * 128 : kt * 256 + qt * 128 + 128]
                            nc.tensor.matmul(
                                op0[:, 0:384],
                                lhsT,
                                xns[:, kt, 0:384],
                                start=(kt == 0),
                                stop=(kt == KT - 1),
                            )
                            nc.tensor.matmul(
                                op1[:, 0:388],
                                lhsT,
                                xns[:, kt, 384:772],
                                start=(kt == 0),
                                stop=(kt == KT - 1),
                            )
                        dst = osb[:, sl * KT + qt]
                        if qt:
                            nc.scalar.copy(dst[:, 0:384], op0[:, 0:384])
                            nc.vector.tensor_copy(dst[:, 384:769], op1[:, 0:385])
                        else:
                            nc.vector.tensor_copy(dst[:, 0:384], op0[:, 0:384])
                            nc.scalar.copy(dst[:, 384:769], op1[:, 0:385])

                if bi == len(batches) - 1 and bn == 2:
                    # split the final store across both HWDGE queues: short tail
                    nc.sync.dma_start(
                        out=y[:, s0].rearrange("p t d -> p (t d)"),
                        in_=osb[:, 0:KT].rearrange("p t d -> p (t d)"),
                    )
                    nc.scalar.dma_start(
                        out=y[:, s0 + 1].rearrange("p t d -> p (t d)"),
                        in_=osb[:, KT : 2 * KT].rearrange("p t d -> p (t d)"),
                    )
                else:
                    # SWDGE: separate queue, never blocks the input rings
                    nc.gpsimd.dma_start(
                        out=y[:, s0 : s0 + bn].rearrange("p s t d -> p (s t) d"),
                        in_=osb[:, 0:TB],
                    )
    return nc


def split_excess_waits(nc, max_waits=1):
    """This walrus build only encodes one sync wait per instruction; move
    excess waits onto preceding same-engine NOPs."""
    n_split = 0
    for fn in nc.m.functions:
        for blk in fn.blocks:
            insts = blk.instructions
            i = 0
            while i < len(insts):
                inst = insts[i]
                si = getattr(inst, "sync_info", None)
                waits = list(si.on_wait) if si and si.on_wait else []
                if len(waits) > max_waits:
                    nop = mybir.InstNoOp(name=f"I-waitsplit-{n_split}", ins=[], outs=[])
                    nop.engine = inst.engine
                    nop.sync_info = mybir.SyncInfo(
                        on_wait=waits[:max_waits], on_update=[]
                    )
                    inst.sync_info = mybir.SyncInfo(
                        on_wait=waits[max_waits:], on_update=list(si.on_update)
                    )
                    insts.insert(i, nop)
                    n_split += 1
                else:
                    i += 1
    return n_split


_NC = None


def _get_nc():
    global _NC
    if _NC is None:
        _NC = build_nc()
        split_excess_waits(_NC)
    return _NC


def shard_inputs(x):
    """Full x [4, 16384, 768] f32 -> 8 per-core dicts:
    x  [128, 16, 2, 772] bf16 (position-major + ones cols)
    xt [128, 16, 6, 2, 128] fp8e4m3 (feature-major)
    """
    xd = np.asarray(x).reshape(B, S_FULL // SEG, SEG, D)[:, :, ::DIL, :]
    xd = xd.reshape(NSEG, KT, 128, D)                 # [seg, t, p, d]
    xp = xd.transpose(2, 0, 1, 3)                     # [p, seg, t, d]
    xb = np.empty((128, NSEG, KT, DW), dtype=ml_dtypes.bfloat16)
    xb[..., 0:D] = xp.astype(ml_dtypes.bfloat16)
    xb[..., D:DW] = np.asarray(1.0, dtype=ml_dtypes.bfloat16)
    xt = (
        xb[..., 0:D]
        .reshape(128, NSEG, KT, DT // 2, 2, 128)      # [p, seg, t, j, c, dp]
        .transpose(5, 1, 3, 4, 2, 0)                  # [dp, seg, j, c, t, p]
        .reshape(128, NSEG, DT // 2, 2, L)            # [dp, seg, j, c, pos]
        .astype(ml_dtypes.float8_e4m3)
    )
    out = []
    for c in range(NCORE):
        sl = slice(SEG_PER_CORE * c, SEG_PER_CORE * (c + 1))
        out.append(
            {
                "x": np.ascontiguousarray(xb[:, sl]),
                "xt": np.ascontiguousarray(xt[:, sl]),
            }
        )
    return out


def assemble_output(results):
    ys = np.concatenate([results[c]["y"] for c in range(NCORE)], axis=1)
    ys = ys.astype(np.float32)                        # [p, seg, t, 769]
    num = ys[..., 0:D].transpose(1, 2, 0, 3)          # [seg, t, p, d]
    den = ys[..., D].transpose(1, 2, 0)[..., None]    # [seg, t, p, 1]
    out = num / den
    return np.ascontiguousarray(out.reshape(B, (S_FULL // SEG) * L, D))


def kernel(x):
    nc = _get_nc()
    in_maps = shard_inputs(x)
    core_ids = list(range(NCORE))
    # run twice: the first execution after a fresh NEFF load has been seen
    # returning unwritten output buffers; the repeat is cheap and reliable.
    run_bass_kernel_spmd(nc, in_maps, core_ids)
    res = run_bass_kernel_spmd(nc, in_maps, core_ids)
    return assemble_output(res.results)
